# revision 10
# baseline (speedup 1.0000x reference)
"""Trainium2 Bass kernel: Mistral flash-attention block with mixed-precision KV cache.

Sharding: tensor-parallel over heads across 8 NeuronCores. Core c owns
q-heads 4c..4c+3 and kv-head c. Each head's attention output is AllGathered
(4 per-head collectives, pipelined) and each core computes a 512-wide hidden
slice of the output projection; the host concatenates the slices.

Layout strategy (per core):
  - All GEMMs run f16 on the PE array (1 cyc/moving-row).
  - Scores are computed TRANSPOSED: sT[kv, seq] = K_T^T(stationary) @ qT(moving),
    so softmax'd weights feed attn@V and o_proj directly with zero on-chip
    transposes. Softmax max-subtraction is replaced by a constant -7 shift
    (raw scores reach ~|180|; shift-invariance makes this exact), the
    1/sqrt(HD) scale is folded into exp's free affine, the causal mask is
    a 0/1 multiply after exp, and the denominator is computed from
    chunk-quad pre-sums (pair adds on GPSIMD, quad adds on DVE) so the
    ones-matmul runs once per 4 chunks instead of per chunk.
  - int4 qdq of the past KV: K on DVE, V on GPSIMD, fully overlapped with
    the q/k/v projections. DMA is split across both HWDGE rings (weights
    on the SP ring, past-KV + tables on the ACT ring) so the first matmul
    issues ~4us in; a burst of dummy matmuls at t=0 warms the PE clock.
"""
import os
import numpy as np

N_CORES = 8
QL, HID, NH, NKV, HD, PAST = 512, 4096, 32, 8, 128, 3584
KV = PAST + QL              # 4096
NHC = NH // N_CORES         # 4 q-heads per core
GS = 32
NCH = KV // 128             # 32 kv chunks
NQCH = PAST // 128          # 28 quantized (past) chunks
NPAIR = NCH // 2            # 16 chunk pairs per head
INV_SQRT_HD = float(1.0 / np.sqrt(128.0))

_CACHE = {}


def _build():
    import concourse.tile as tile
    from concourse import bacc, mybir

    f32 = mybir.dt.float32
    i32 = mybir.dt.int32
    f16 = mybir.dt.float16
    AF = mybir.ActivationFunctionType
    AL = mybir.AluOpType

    nc = bacc.Bacc("TRN2", target_bir_lowering=False, debug=False,
                   num_devices=N_CORES)

    HIDT = nc.dram_tensor("hidt", [HID, QL], f16, kind="ExternalInput")
    WQT = nc.dram_tensor("wqt", [HID, NHC * HD], f16, kind="ExternalInput")
    WKT = nc.dram_tensor("wkt", [HID, HD], f16, kind="ExternalInput")
    WVT = nc.dram_tensor("wvt", [HID, HD], f16, kind="ExternalInput")
    WOT = nc.dram_tensor("wot", [NH * HD, QL], f16, kind="ExternalInput")
    PKT = nc.dram_tensor("pkt", [HD, PAST], f32, kind="ExternalInput")
    PV = nc.dram_tensor("pv", [PAST, HD], f32, kind="ExternalInput")
    COST = nc.dram_tensor("cost", [HD, QL], f32, kind="ExternalInput")
    SINTS = nc.dram_tensor("sints", [HD, QL], f32, kind="ExternalInput")
    MASKP = nc.dram_tensor("maskp", [128, 4 * QL], f16, kind="ExternalInput")
    ONES = nc.dram_tensor("ones", [HD, 1], f16, kind="ExternalInput")
    OUT = nc.dram_tensor("out", [QL, QL], f32, kind="ExternalOutput")
    agin_h = [nc.dram_tensor(f"agin_{h}", [128, QL], f16) for h in range(NHC)]
    agout_h = [nc.dram_tensor(f"agout_{h}", [N_CORES * 128, QL], f16,
                              addr_space="Shared") for h in range(NHC)]

    rg = [list(range(N_CORES))]

    with tile.TileContext(nc) as tc:
        pconst_cm = tc.tile_pool(name="pconst", bufs=1)
        pconst = pconst_cm.__enter__()
        kt_all = pconst.tile([128, KV], f16, tag="kt_all")
        v_all = pconst.tile([128, NCH * HD], f16, tag="v_all")
        cosT = pconst.tile([128, QL], f32, tag="cosT")
        sinTs = pconst.tile([128, QL], f32, tag="sinTs")
        ones = pconst.tile([128, 1], f16, tag="ones")
        masks = pconst.tile([128, 4 * QL], f16, tag="masks")
        nbias = pconst.tile([128, 1], f32, tag="nbias")
        wtile = pconst.tile([128, QL], f16, tag="wtile")

        # ACT-ring DMAs (qActDynamicHW): past KV + small tables, so the
        # weight stream on the SP ring is never queued behind them.
        pqdq_cm = tc.tile_pool(name="pqdq", bufs=1)
        pq = pqdq_cm.__enter__()
        pk = pq.tile([128, PAST], f32, tag="ksrc")
        pvt = pq.tile([128, PAST], f32, tag="vsrc")
        nc.scalar.dma_start(pk[:], PKT[:])
        nc.scalar.dma_start(cosT[:], COST[:])
        nc.scalar.dma_start(sinTs[:], SINTS[:])
        nc.scalar.dma_start(ones[:], ONES[:])
        nc.scalar.dma_start(
            pvt[:].rearrange("p (c h) -> p c h", h=HD),
            PV[:].rearrange("(c p) h -> p c h", p=128))
        nc.scalar.dma_start(masks[:], MASKP[:])

        nc.vector.memset(wtile[:], 0.0)
        nc.vector.tensor_scalar_mul(nbias[:], ones[:], -7.0)

        pqkv_cm = tc.tile_pool(name="pqkv", bufs=1)
        pqkv = pqkv_cm.__enter__()
        qt_sb = [pqkv.tile([128, QL], f16, tag=f"qt{h}", name=f"qt_sb{h}")
                 for h in range(NHC)]

        with tc.tile_pool(name="pstream", bufs=3) as pstr, \
             tc.tile_pool(name="ptmp", bufs=2) as ptmp, \
             tc.tile_pool(name="ps_qkv", bufs=1, space="PSUM") as ps_qkv:

            # PE clock warm-up: ~12 dummy matmuls (~3.5us cold) so the HAM
            # un-throttles before the first real projection matmul.
            ps_w = ps_qkv.tile([128, QL], f32, tag="warm")
            for _ in range(12):
                nc.tensor.matmul(ps_w[:], wtile[:, 0:128], wtile[:],
                                 start=True, stop=True)
            # dummy reader so the BIR verifier accepts the warm-up writes
            nc.vector.tensor_copy(wtile[0:1, 0:1], ps_w[0:1, 0:1])

            # ---------------- qdq of the past KV cache ----------------
            # eng=engine for the big elementwise passes; the tiny per-group
            # scale math stays on DVE (reciprocal is DVE-only).
            def qdq(eng, src_ap_2d, out_grouped, n, tag):
                ngr = n // GS
                mn = pq.tile([128, ngr], f32, tag=f"mn{tag}")
                mx = pq.tile([128, ngr], f32, tag=f"mx{tag}")
                g_in = src_ap_2d.rearrange("p (g i) -> p g i", i=GS)
                # free-dim reduces are DVE-only; the big elementwise passes
                # below run on `eng` (GPSIMD for V, DVE for K)
                nc.vector.tensor_reduce(mn[:], g_in, mybir.AxisListType.X, AL.min)
                nc.vector.tensor_reduce(mx[:], g_in, mybir.AxisListType.X, AL.max)
                sc = pq.tile([128, ngr], f32, tag=f"sc{tag}")
                nc.vector.tensor_sub(sc[:], mx[:], mn[:])
                nc.vector.tensor_scalar_mul(sc[:], sc[:], 1.0 / 15.0)
                scl = pq.tile([128, ngr], f32, tag=f"scl{tag}")
                nc.vector.tensor_scalar_max(scl[:], sc[:], 1e-30)
                rs = pq.tile([128, ngr], f32, tag=f"rs{tag}")
                nc.vector.reciprocal(rs[:], scl[:])
                mnb = mn[:].unsqueeze(2).broadcast_to((128, ngr, GS))
                rsb = rs[:].unsqueeze(2).broadcast_to((128, ngr, GS))
                scb = sc[:].unsqueeze(2).broadcast_to((128, ngr, GS))
                t1 = pq.tile([128, n], f32, tag=f"t1{tag}")
                t1g = t1[:].rearrange("p (g i) -> p g i", i=GS)
                eng.tensor_sub(t1g, g_in, mnb)           # x - mn
                t2 = pq.tile([128, n], f32, tag=f"t2{tag}")
                t2g = t2[:].rearrange("p (g i) -> p g i", i=GS)
                eng.tensor_mul(t2g, t1g, rsb)            # u = (x-mn)*rs
                ti = pq.tile([128, n], i32, tag=f"ti{tag}")
                eng.tensor_copy(ti[:], t2[:])            # round-half-even
                eng.tensor_copy(t1[:], ti[:])            # back to f32
                eng.tensor_mul(t2g, t1g, scb)            # q * scale
                eng.tensor_add(out_grouped, t2g, mnb)    # + mn -> f16 out

            qdq(nc.vector, pk[:],
                kt_all[:, 0:PAST].rearrange("p (g i) -> p g i", i=GS),
                PAST, "k")
            qdq(nc.gpsimd, pvt[:],
                v_all[:, 0:PAST].rearrange("p (g i) -> p g i", i=GS),
                PAST, "v")

            # ---------------- q/k/v projections (PE) ----------------
            # Per-chunk PE order [k, v, q0..q3] so kt finishes earliest and
            # its RoPE (DVE) runs under the tail of the projection matmuls.
            qt_ps = [ps_qkv.tile([128, QL], f32, tag=f"qps{h}", name=f"qt_ps{h}")
                     for h in range(NHC)]
            kt_ps = ps_qkv.tile([128, QL], f32, tag="kps")
            v_ps = ps_qkv.tile([128, QL], f32, tag="vps")
            NK2 = HID // 256
            wk4 = wv4 = None
            for k2 in range(NK2):
                hid2 = pstr.tile([128, 2, QL], f16, tag="hid")
                nc.sync.dma_start(
                    hid2[:],
                    HIDT[k2 * 256:(k2 + 1) * 256, :].rearrange(
                        "(a p) q -> p a q", p=128))
                wq2 = pstr.tile([128, 2, NHC * HD], f16, tag="wq")
                nc.sync.dma_start(
                    wq2[:],
                    WQT[k2 * 256:(k2 + 1) * 256, :].rearrange(
                        "(a p) q -> p a q", p=128))
                if k2 % 2 == 0:
                    wk4 = pstr.tile([128, 4, HD], f16, tag="wk")
                    nc.sync.dma_start(
                        wk4[:],
                        WKT[k2 * 256:(k2 + 2) * 256, :].rearrange(
                            "(a p) q -> p a q", p=128))
                    wv4 = pstr.tile([128, 4, HD], f16, tag="wv")
                    nc.sync.dma_start(
                        wv4[:],
                        WVT[k2 * 256:(k2 + 2) * 256, :].rearrange(
                            "(a p) q -> p a q", p=128))
                for a in range(2):
                    k = 2 * k2 + a
                    aq = k % 4
                    st, sp = (k == 0), (k == 2 * NK2 - 1)
                    nc.tensor.matmul(kt_ps[:], wk4[:, aq, :], hid2[:, a, :],
                                     start=st, stop=sp)
                    # all four seq-chunk groups share one PSUM bank: only the
                    # first matmul clears it (start=True wipes the WHOLE bank)
                    for s in range(4):
                        mm = nc.tensor.matmul(
                            v_ps[:, s * 128:(s + 1) * 128],
                            hid2[:, a, s * 128:(s + 1) * 128], wv4[:, aq, :],
                            start=(st and s == 0), stop=sp,
                            skip_group_check=True)
                        if st and s == 0:
                            v_mm0 = mm
                        elif st:
                            tile.add_dep_helper(
                                mm.ins, v_mm0.ins, sync=False,
                                reason="bank clear before first writes")
                    for h in range(NHC):
                        nc.tensor.matmul(qt_ps[h][:],
                                         wq2[:, a, h * 128:(h + 1) * 128],
                                         hid2[:, a, :], start=st, stop=sp)

            # RoPE on kT / qT (DVE + 2 partition-shift DMAs each)
            def rope(ps, out_ap):
                xsb = ptmp.tile([128, QL], f32, tag="xsb")
                nc.vector.tensor_copy(xsb[:], ps[:])
                tcos = ptmp.tile([128, QL], f32, tag="tcos")
                nc.vector.tensor_mul(tcos[:], xsb[:], cosT[:])
                rot = ptmp.tile([128, QL], f32, tag="rot")
                nc.scalar.dma_start(rot[0:64, :], xsb[64:128, :])
                nc.scalar.dma_start(rot[64:128, :], xsb[0:64, :])
                nc.vector.tensor_mul(rot[:], rot[:], sinTs[:])
                nc.vector.tensor_add(out_ap, tcos[:], rot[:])

            rope(kt_ps[:], kt_all[:, PAST:KV])
            rope(qt_ps[0][:], qt_sb[0][:])
            # new V -> cache chunks 28..31 (one copy, f16 rounding on write);
            # needed only after head 0's first exp, so it sits between ropes
            nc.vector.tensor_copy(v_all[:, NQCH * HD:NCH * HD], v_ps[:])
            for h in range(1, NHC):
                rope(qt_ps[h][:], qt_sb[h][:])

        # prefetch the whole o_proj weight slice during attention (SP ring)
        pwot_cm = tc.tile_pool(name="pwot", bufs=1)
        pwot = pwot_cm.__enter__()
        wot_tiles = []
        for g in range(NH):
            wt = pwot.tile([128, QL], f16, tag=f"wot{g}", name=f"wot{g}")
            nc.sync.dma_start(wt[:], WOT[g * 128:(g + 1) * 128, :])
            wot_tiles.append(wt)

        # ---------------- attention, head by head ----------------
        with tc.tile_pool(name="pexp", bufs=3) as pexp, \
             tc.tile_pool(name="ppair", bufs=4) as ppair, \
             tc.tile_pool(name="pquad", bufs=2) as pquad, \
             tc.tile_pool(name="pmisc", bufs=2) as pmisc, \
             tc.tile_pool(name="ps_s", bufs=2, space="PSUM") as ps_s, \
             tc.tile_pool(name="ps_u", bufs=2, space="PSUM") as ps_u, \
             tc.tile_pool(name="ps_d", bufs=1, space="PSUM") as ps_d:
            for h in range(NHC):
                outU = ps_u.tile([128, QL], f32, tag="outU")
                den = ps_d.tile([1, QL], f32, tag="den")
                epairs = [None] * NPAIR
                psums = [None] * NPAIR
                qsums = [None] * (NPAIR // 2)

                def emit_scores(j):
                    s_ps = ps_s.tile([128, 2 * QL], f32, tag="score",
                                     name=f"s_ps{h}_{j}")
                    for a in range(2):
                        c = 2 * j + a
                        off = (c - NQCH) * 128 if c >= NQCH else 0
                        nc.tensor.matmul(
                            s_ps[:, a * QL + off:(a + 1) * QL],
                            kt_all[:, c * 128:(c + 1) * 128],
                            qt_sb[h][:, off:QL], start=True, stop=True,
                            skip_group_check=(off > 0))
                    e = pexp.tile([128, 2 * QL], f16, tag="e")
                    # constant shift keeps exp within f16 range (raw scores
                    # reach ~|180|); softmax is shift-invariant so num/den
                    # cancel exactly.
                    nc.scalar.activation(e[:], s_ps[:], AF.Exp,
                                         scale=INV_SQRT_HD, bias=nbias[:])
                    if j >= NQCH // 2:
                        moff = (j - NQCH // 2) * 2 * QL
                        nc.vector.tensor_mul(
                            e[:], e[:], masks[:, moff:moff + 2 * QL])
                    epairs[j] = e
                    # chunk-pair sum for the softmax denominator (GPSIMD)
                    p = ppair.tile([128, QL], f16, tag="pair",
                                   name=f"pair{h}_{j}")
                    nc.gpsimd.tensor_add(p[:], e[:, 0:QL], e[:, QL:2 * QL])
                    psums[j] = p
                    if j % 2 == 1:
                        q = pquad.tile([128, QL], f16, tag="quad",
                                       name=f"quad{h}_{j // 2}")
                        nc.vector.tensor_add(q[:], psums[j - 1][:], psums[j][:])
                        qsums[j // 2] = q

                def emit_attn(jj):
                    for a in range(2):
                        c = 2 * jj + a
                        off = (c - NQCH) * 128 if c >= NQCH else 0
                        ea = epairs[jj][:, a * QL + off:(a + 1) * QL]
                        nc.tensor.matmul(outU[:, off:QL],
                                         v_all[:, c * HD:(c + 1) * HD],
                                         ea, start=(c == 0),
                                         stop=(c == NCH - 1),
                                         skip_group_check=(off > 0))

                def emit_den(t):
                    nc.tensor.matmul(den[:], ones[:], qsums[t][:],
                                     start=(t == 0), stop=(t == NPAIR // 2 - 1))

                for j in range(NPAIR):
                    emit_scores(j)
                    if j >= 1:
                        emit_attn(j - 1)
                    if j >= 3 and j % 2 == 1:
                        emit_den((j - 3) // 2)
                emit_attn(NPAIR - 1)
                emit_den(NPAIR // 2 - 1)

                den_sb = pmisc.tile([1, QL], f32, tag="den_sb")
                nc.vector.tensor_copy(den_sb[:], den[:])
                rden = pmisc.tile([1, QL], f32, tag="rden")
                nc.vector.reciprocal_approx_fast(rden[:], den_sb[:])
                bc = pmisc.tile([128, QL], f32, tag="bc")
                nc.gpsimd.partition_broadcast(bc[:], rden[:])
                outT = pmisc.tile([128, QL], f16, tag="outT")
                nc.vector.tensor_mul(outT[:], outU[:], bc[:])

                nc.scalar.dma_start(agin_h[h][:], outT[:])
                nc.gpsimd.collective_compute(
                    "AllGather", mybir.AluOpType.bypass, replica_groups=rg,
                    ins=[agin_h[h][:]], outs=[agout_h[h][:]])

        # ---------------- output projection over the 512-wide hid slice ----------------
        with tc.tile_pool(name="poproj", bufs=4) as po, \
             tc.tile_pool(name="ps_o", bufs=1, space="PSUM") as ps_o:
            o_ps = [ps_o.tile([128, QL], f32, tag=f"o{s}", name=f"o_ps{s}")
                    for s in range(4)]
            n_blk = NHC * N_CORES
            bi = 0
            for h in range(NHC):
                for cp in range(N_CORES):
                    g = NHC * cp + h
                    agt = po.tile([128, QL], f16, tag="agt")
                    nc.sync.dma_start(
                        agt[:], agout_h[h][cp * 128:(cp + 1) * 128, :])
                    st, sp = (bi == 0), (bi == n_blk - 1)
                    for s in range(4):
                        nc.tensor.matmul(o_ps[s][:],
                                         agt[:, s * 128:(s + 1) * 128],
                                         wot_tiles[g][:], start=st, stop=sp)
                    bi += 1
            for s in range(4):
                osb = po.tile([128, QL], f32, tag="osb")
                nc.vector.tensor_copy(osb[:], o_ps[s][:])
                nc.sync.dma_start(OUT[s * 128:(s + 1) * 128, :], osb[:])

        pwot_cm.__exit__(None, None, None)
        pqkv_cm.__exit__(None, None, None)
        pqdq_cm.__exit__(None, None, None)
        pconst_cm.__exit__(None, None, None)

    nc.compile()
    return nc


def _host_prep(inputs):
    hid = np.asarray(inputs["hidden_states"], dtype=np.float32)[0]   # [512, 4096]
    wq = np.asarray(inputs["wq"], dtype=np.float32)
    wk = np.asarray(inputs["wk"], dtype=np.float32)
    wv = np.asarray(inputs["wv"], dtype=np.float32)
    wo = np.asarray(inputs["wo"], dtype=np.float32)
    pk = np.asarray(inputs["past_key"], dtype=np.float32)[0]         # [8, 3584, 128]
    pv = np.asarray(inputs["past_value"], dtype=np.float32)[0]
    pos = np.asarray(inputs["position_ids"])[0].astype(np.float32)   # [512]

    hidT = np.ascontiguousarray(hid.T)
    inv_freq = np.float32(1.0) / (
        np.float32(10000.0) ** (np.arange(0, HD, 2, dtype=np.float32)
                                / np.float32(HD)))
    freqs = (pos[:, None] * inv_freq[None, :]).astype(np.float32)    # [512, 64]
    emb = np.concatenate([freqs, freqs], axis=-1).astype(np.float64)
    cosT = np.ascontiguousarray(np.cos(emb).astype(np.float32).T)    # [128, 512]
    sinT = np.ascontiguousarray(np.sin(emb).astype(np.float32).T)
    sinTs = sinT.copy()
    sinTs[0:64] *= np.float32(-1.0)
    mask = (np.arange(QL)[:, None] <= np.arange(QL)[None, :]).astype(np.float32)
    # device layout: [partition, (mask-chunk, seq)]
    maskp = np.ascontiguousarray(
        mask.reshape(4, 128, QL).transpose(1, 0, 2).reshape(128, 4 * QL)
    ).astype(np.float16)
    ones = np.ones((HD, 1), np.float16)

    hidT16 = hidT.astype(np.float16)
    in_maps = []
    for c in range(N_CORES):
        in_maps.append({
            "hidt": hidT16,
            "wqt": np.ascontiguousarray(wq[c * 512:(c + 1) * 512, :].T).astype(np.float16),
            "wkt": np.ascontiguousarray(wk[c * 128:(c + 1) * 128, :].T).astype(np.float16),
            "wvt": np.ascontiguousarray(wv[c * 128:(c + 1) * 128, :].T).astype(np.float16),
            "wot": np.ascontiguousarray(wo[c * 512:(c + 1) * 512, :].T).astype(np.float16),
            "pkt": np.ascontiguousarray(pk[c].T),
            "pv": np.ascontiguousarray(pv[c]),
            "cost": cosT,
            "sints": sinTs,
            "maskp": maskp,
            "ones": ones,
        })
    return in_maps


def _run(inputs, trace=False):
    from concourse.bass_utils import run_bass_kernel_spmd
    if "nc" not in _CACHE:
        _CACHE["nc"] = _build()
    nc = _CACHE["nc"]
    in_maps = _host_prep(inputs)
    res = run_bass_kernel_spmd(nc, in_maps, list(range(N_CORES)), trace=trace)
    out = np.concatenate([res.results[c]["out"] for c in range(N_CORES)], axis=1)
    return out.reshape(1, QL, HID).astype(np.float32), res


def kernel(**inputs) -> np.ndarray:
    out, _ = _run(inputs, trace=False)
    return out


# revision 14
# speedup vs baseline: 1.1344x; 1.1344x over previous
"""Trainium2 Bass kernel: Mistral flash-attention block with mixed-precision KV cache.

Sharding: tensor-parallel over heads across 8 NeuronCores. Core c owns
q-heads 4c..4c+3 and kv-head c. Each head's attention output is AllGathered
(4 per-head collectives, pipelined) and each core computes a 512-wide hidden
slice of the output projection; the host concatenates the slices.

Layout strategy (per core):
  - All GEMMs run f16 on the PE array (1 cyc/moving-row).
  - Scores are computed TRANSPOSED: sT[kv, seq] = K_T^T(stationary) @ qT(moving),
    so softmax'd weights feed attn@V and o_proj directly with zero on-chip
    transposes. Softmax max-subtraction is replaced by a constant -7 shift
    (raw scores reach ~|180|; shift-invariance makes this exact), the
    1/sqrt(HD) scale is folded into exp's free affine, the causal mask is
    a 0/1 multiply after exp, and the denominator is computed from
    chunk-quad pre-sums (pair adds on GPSIMD, quad adds on DVE) so the
    ones-matmul runs once per 4 chunks instead of per chunk.
  - int4 qdq of the past KV: K on DVE, V on GPSIMD, fully overlapped with
    the q/k/v projections. DMA is split across both HWDGE rings (weights
    on the SP ring, past-KV + tables on the ACT ring) so the first matmul
    issues ~4us in; a burst of dummy matmuls at t=0 warms the PE clock.
"""
import os
import numpy as np

N_CORES = 8
QL, HID, NH, NKV, HD, PAST = 512, 4096, 32, 8, 128, 3584
KV = PAST + QL              # 4096
NHC = NH // N_CORES         # 4 q-heads per core
GS = 32
NCH = KV // 128             # 32 kv chunks
NQCH = PAST // 128          # 28 quantized (past) chunks
NPAIR = NCH // 2            # 16 chunk pairs per head
INV_SQRT_HD = float(1.0 / np.sqrt(128.0))

_CACHE = {}


def _build():
    import concourse.tile as tile
    from concourse import bacc, mybir

    f32 = mybir.dt.float32
    i32 = mybir.dt.int32
    f16 = mybir.dt.float16
    AF = mybir.ActivationFunctionType
    AL = mybir.AluOpType

    nc = bacc.Bacc("TRN2", target_bir_lowering=False, debug=False,
                   num_devices=N_CORES)

    HIDT = nc.dram_tensor("hidt", [HID, QL], f16, kind="ExternalInput")
    WQT = nc.dram_tensor("wqt", [HID, NHC * HD], f16, kind="ExternalInput")
    WKT = nc.dram_tensor("wkt", [HID, HD], f16, kind="ExternalInput")
    WVT = nc.dram_tensor("wvt", [HID, HD], f16, kind="ExternalInput")
    WOT = nc.dram_tensor("wot", [NH * HD, QL], f16, kind="ExternalInput")
    PKT = nc.dram_tensor("pkt", [HD, PAST], f32, kind="ExternalInput")
    PV = nc.dram_tensor("pv", [PAST, HD], f32, kind="ExternalInput")
    COST = nc.dram_tensor("cost", [HD, QL], f32, kind="ExternalInput")
    SINTS = nc.dram_tensor("sints", [HD, QL], f32, kind="ExternalInput")
    MASKP = nc.dram_tensor("maskp", [128, 4 * QL], f16, kind="ExternalInput")
    ONES = nc.dram_tensor("ones", [HD, 1], f16, kind="ExternalInput")
    OUT = nc.dram_tensor("out", [QL, QL], f32, kind="ExternalOutput")
    agin_ab = [nc.dram_tensor(f"agin_{p}", [2 * 128, QL], f16)
               for p in range(2)]
    agout_ab = [nc.dram_tensor(f"agout_{p}", [N_CORES * 2 * 128, QL], f16,
                               addr_space="Shared") for p in range(2)]

    rg = [list(range(N_CORES))]

    with tile.TileContext(nc) as tc:
        pconst_cm = tc.tile_pool(name="pconst", bufs=1)
        pconst = pconst_cm.__enter__()
        kt_all = pconst.tile([128, KV], f16, tag="kt_all")
        v_all = pconst.tile([128, NCH * HD], f16, tag="v_all")
        cosT = pconst.tile([128, QL], f32, tag="cosT")
        sinTs = pconst.tile([128, QL], f32, tag="sinTs")
        ones = pconst.tile([128, 1], f16, tag="ones")
        masks = pconst.tile([128, 4 * QL], f16, tag="masks")
        nbias = pconst.tile([128, 1], f32, tag="nbias")
        wtile = pconst.tile([128, QL], f16, tag="wtile")

        # ACT-ring DMAs (qActDynamicHW): past KV + small tables, so the
        # weight stream on the SP ring is never queued behind them.
        pqdq_cm = tc.tile_pool(name="pqdq", bufs=1)
        pq = pqdq_cm.__enter__()
        pk = pq.tile([128, PAST], f32, tag="ksrc")
        pvt = pq.tile([128, PAST], f32, tag="vsrc")
        nc.scalar.dma_start(pk[:], PKT[:])
        nc.scalar.dma_start(cosT[:], COST[:])
        nc.scalar.dma_start(sinTs[:], SINTS[:])
        nc.scalar.dma_start(ones[:], ONES[:])
        nc.scalar.dma_start(
            pvt[:].rearrange("p (c h) -> p c h", h=HD),
            PV[:].rearrange("(c p) h -> p c h", p=128))
        nc.scalar.dma_start(masks[:], MASKP[:])

        nc.vector.memset(wtile[:], 0.0)
        nc.vector.tensor_scalar_mul(nbias[:], ones[:], -7.0)

        pqkv_cm = tc.tile_pool(name="pqkv", bufs=1)
        pqkv = pqkv_cm.__enter__()
        qt_sb = [pqkv.tile([128, QL], f16, tag=f"qt{h}", name=f"qt_sb{h}")
                 for h in range(NHC)]

        with tc.tile_pool(name="pstream", bufs=3) as pstr, \
             tc.tile_pool(name="ptmp", bufs=2) as ptmp, \
             tc.tile_pool(name="ps_qkv", bufs=1, space="PSUM") as ps_qkv:

            # PE clock warm-up: ~12 dummy matmuls (~3.5us cold) so the HAM
            # un-throttles before the first real projection matmul.
            ps_w = ps_qkv.tile([128, QL], f32, tag="warm")
            for _ in range(12):
                nc.tensor.matmul(ps_w[:], wtile[:, 0:128], wtile[:],
                                 start=True, stop=True)
            # dummy reader so the BIR verifier accepts the warm-up writes
            nc.vector.tensor_copy(wtile[0:1, 0:1], ps_w[0:1, 0:1])

            # ---------------- qdq of the past KV cache ----------------
            # eng=engine for the big elementwise passes; the tiny per-group
            # scale math stays on DVE (reciprocal is DVE-only).
            def qdq(eng, src_ap_2d, out_grouped, n, tag):
                ngr = n // GS
                mn = pq.tile([128, ngr], f32, tag=f"mn{tag}")
                mx = pq.tile([128, ngr], f32, tag=f"mx{tag}")
                g_in = src_ap_2d.rearrange("p (g i) -> p g i", i=GS)
                # free-dim reduces are DVE-only; the big elementwise passes
                # below run on `eng` (GPSIMD for V, DVE for K)
                nc.vector.tensor_reduce(mn[:], g_in, mybir.AxisListType.X, AL.min)
                nc.vector.tensor_reduce(mx[:], g_in, mybir.AxisListType.X, AL.max)
                sc = pq.tile([128, ngr], f32, tag=f"sc{tag}")
                nc.vector.tensor_sub(sc[:], mx[:], mn[:])
                nc.vector.tensor_scalar_mul(sc[:], sc[:], 1.0 / 15.0)
                scl = pq.tile([128, ngr], f32, tag=f"scl{tag}")
                nc.vector.tensor_scalar_max(scl[:], sc[:], 1e-30)
                rs = pq.tile([128, ngr], f32, tag=f"rs{tag}")
                nc.vector.reciprocal(rs[:], scl[:])
                mnb = mn[:].unsqueeze(2).broadcast_to((128, ngr, GS))
                rsb = rs[:].unsqueeze(2).broadcast_to((128, ngr, GS))
                scb = sc[:].unsqueeze(2).broadcast_to((128, ngr, GS))
                t1 = pq.tile([128, n], f32, tag=f"t1{tag}")
                t1g = t1[:].rearrange("p (g i) -> p g i", i=GS)
                eng.tensor_sub(t1g, g_in, mnb)           # x - mn
                t2 = pq.tile([128, n], f32, tag=f"t2{tag}")
                t2g = t2[:].rearrange("p (g i) -> p g i", i=GS)
                eng.tensor_mul(t2g, t1g, rsb)            # u = (x-mn)*rs
                ti = pq.tile([128, n], i32, tag=f"ti{tag}")
                eng.tensor_copy(ti[:], t2[:])            # round-half-even
                eng.tensor_copy(t1[:], ti[:])            # back to f32
                eng.tensor_mul(t2g, t1g, scb)            # q * scale
                eng.tensor_add(out_grouped, t2g, mnb)    # + mn -> f16 out

            # both on DVE: GPSIMD tensor ops measure ~8x slower than DVE
            qdq(nc.vector, pk[:],
                kt_all[:, 0:PAST].rearrange("p (g i) -> p g i", i=GS),
                PAST, "k")
            qdq(nc.vector, pvt[:],
                v_all[:, 0:PAST].rearrange("p (g i) -> p g i", i=GS),
                PAST, "v")

            # ---------------- q/k/v projections (PE) ----------------
            # Per-chunk PE order [k, v, q0..q3] so kt finishes earliest and
            # its RoPE (DVE) runs under the tail of the projection matmuls.
            qt_ps = [ps_qkv.tile([128, QL], f32, tag=f"qps{h}", name=f"qt_ps{h}")
                     for h in range(NHC)]
            kt_ps = ps_qkv.tile([128, QL], f32, tag="kps")
            v_ps = ps_qkv.tile([128, QL], f32, tag="vps")
            NK2 = HID // 256
            wk4 = wv4 = None
            for k2 in range(NK2):
                hid2 = pstr.tile([128, 2, QL], f16, tag="hid")
                nc.sync.dma_start(
                    hid2[:],
                    HIDT[k2 * 256:(k2 + 1) * 256, :].rearrange(
                        "(a p) q -> p a q", p=128))
                wq2 = pstr.tile([128, 2, NHC * HD], f16, tag="wq")
                nc.sync.dma_start(
                    wq2[:],
                    WQT[k2 * 256:(k2 + 1) * 256, :].rearrange(
                        "(a p) q -> p a q", p=128))
                if k2 % 2 == 0:
                    wk4 = pstr.tile([128, 4, HD], f16, tag="wk")
                    nc.sync.dma_start(
                        wk4[:],
                        WKT[k2 * 256:(k2 + 2) * 256, :].rearrange(
                            "(a p) q -> p a q", p=128))
                    wv4 = pstr.tile([128, 4, HD], f16, tag="wv")
                    nc.sync.dma_start(
                        wv4[:],
                        WVT[k2 * 256:(k2 + 2) * 256, :].rearrange(
                            "(a p) q -> p a q", p=128))
                for a in range(2):
                    k = 2 * k2 + a
                    aq = k % 4
                    st, sp = (k == 0), (k == 2 * NK2 - 1)
                    nc.tensor.matmul(kt_ps[:], wk4[:, aq, :], hid2[:, a, :],
                                     start=st, stop=sp)
                    # all four seq-chunk groups share one PSUM bank: only the
                    # first matmul clears it (start=True wipes the WHOLE bank)
                    for s in range(4):
                        mm = nc.tensor.matmul(
                            v_ps[:, s * 128:(s + 1) * 128],
                            hid2[:, a, s * 128:(s + 1) * 128], wv4[:, aq, :],
                            start=(st and s == 0), stop=sp,
                            skip_group_check=True)
                        if st and s == 0:
                            v_mm0 = mm
                        elif st:
                            tile.add_dep_helper(
                                mm.ins, v_mm0.ins, sync=False,
                                reason="bank clear before first writes")
                    for h in range(NHC):
                        nc.tensor.matmul(qt_ps[h][:],
                                         wq2[:, a, h * 128:(h + 1) * 128],
                                         hid2[:, a, :], start=st, stop=sp)

            # RoPE on kT / qT (DVE + 2 partition-shift DMAs each)
            def rope(ps, out_ap):
                xsb = ptmp.tile([128, QL], f32, tag="xsb")
                nc.vector.tensor_copy(xsb[:], ps[:])
                tcos = ptmp.tile([128, QL], f32, tag="tcos")
                nc.vector.tensor_mul(tcos[:], xsb[:], cosT[:])
                rot = ptmp.tile([128, QL], f32, tag="rot")
                nc.scalar.dma_start(rot[0:64, :], xsb[64:128, :])
                nc.scalar.dma_start(rot[64:128, :], xsb[0:64, :])
                nc.vector.tensor_mul(rot[:], rot[:], sinTs[:])
                nc.vector.tensor_add(out_ap, tcos[:], rot[:])

            rope(kt_ps[:], kt_all[:, PAST:KV])
            rope(qt_ps[0][:], qt_sb[0][:])
            # new V -> cache chunks 28..31 (one copy, f16 rounding on write);
            # needed only after head 0's first exp, so it sits between ropes
            nc.vector.tensor_copy(v_all[:, NQCH * HD:NCH * HD], v_ps[:])
            for h in range(1, NHC):
                rope(qt_ps[h][:], qt_sb[h][:])

        # prefetch the whole o_proj weight slice during attention (SP ring)
        pwot_cm = tc.tile_pool(name="pwot", bufs=1)
        pwot = pwot_cm.__enter__()
        wot_tiles = []
        for g in range(NH):
            wt = pwot.tile([128, QL], f16, tag=f"wot{g}", name=f"wot{g}")
            nc.sync.dma_start(wt[:], WOT[g * 128:(g + 1) * 128, :])
            wot_tiles.append(wt)

        # ---------------- attention, head by head ----------------
        with tc.tile_pool(name="pexp", bufs=3) as pexp, \
             tc.tile_pool(name="pmisc", bufs=2) as pmisc, \
             tc.tile_pool(name="ps_s", bufs=2, space="PSUM") as ps_s, \
             tc.tile_pool(name="ps_u", bufs=2, space="PSUM") as ps_u, \
             tc.tile_pool(name="ps_d", bufs=1, space="PSUM") as ps_d:
            for h in range(NHC):
                outU = ps_u.tile([128, QL], f32, tag="outU")
                den = ps_d.tile([1, QL], f32, tag="den")
                epairs = [None] * NPAIR

                def emit_scores(j):
                    s_ps = ps_s.tile([128, 2 * QL], f32, tag="score",
                                     name=f"s_ps{h}_{j}")
                    for a in range(2):
                        c = 2 * j + a
                        off = (c - NQCH) * 128 if c >= NQCH else 0
                        nc.tensor.matmul(
                            s_ps[:, a * QL + off:(a + 1) * QL],
                            kt_all[:, c * 128:(c + 1) * 128],
                            qt_sb[h][:, off:QL], start=True, stop=True,
                            skip_group_check=(off > 0))
                    e = pexp.tile([128, 2 * QL], f16, tag="e")
                    # constant shift keeps exp within f16 range (raw scores
                    # reach ~|180|); softmax is shift-invariant so num/den
                    # cancel exactly.
                    nc.scalar.activation(e[:], s_ps[:], AF.Exp,
                                         scale=INV_SQRT_HD, bias=nbias[:])
                    if j >= NQCH // 2:
                        moff = (j - NQCH // 2) * 2 * QL
                        nc.vector.tensor_mul(
                            e[:], e[:], masks[:, moff:moff + 2 * QL])
                    epairs[j] = e

                def emit_attn(jj):
                    for a in range(2):
                        c = 2 * jj + a
                        off = (c - NQCH) * 128 if c >= NQCH else 0
                        ea = epairs[jj][:, a * QL + off:(a + 1) * QL]
                        nc.tensor.matmul(outU[:, off:QL],
                                         v_all[:, c * HD:(c + 1) * HD],
                                         ea, start=(c == 0),
                                         stop=(c == NCH - 1),
                                         skip_group_check=(off > 0))
                        nc.tensor.matmul(den[:], ones[:],
                                         epairs[jj][:, a * QL:(a + 1) * QL],
                                         start=(c == 0), stop=(c == NCH - 1))

                for j in range(NPAIR):
                    emit_scores(j)
                    if j >= 1:
                        emit_attn(j - 1)
                emit_attn(NPAIR - 1)

                den_sb = pmisc.tile([1, QL], f32, tag="den_sb")
                nc.vector.tensor_copy(den_sb[:], den[:])
                rden = pmisc.tile([1, QL], f32, tag="rden")
                nc.vector.reciprocal_approx_fast(rden[:], den_sb[:])
                bc = pmisc.tile([128, QL], f32, tag="bc")
                nc.gpsimd.partition_broadcast(bc[:], rden[:])
                outT = pmisc.tile([128, QL], f16, tag="outT")
                nc.vector.tensor_mul(outT[:], outU[:], bc[:])

                nc.scalar.dma_start(
                    agin_ab[h // 2][(h % 2) * 128:(h % 2 + 1) * 128, :], outT[:])
                if h % 2 == 1:
                    nc.gpsimd.collective_compute(
                        "AllGather", mybir.AluOpType.bypass, replica_groups=rg,
                        ins=[agin_ab[h // 2][:]], outs=[agout_ab[h // 2][:]])

        # ---------------- output projection over the 512-wide hid slice ----------------
        with tc.tile_pool(name="poproj", bufs=4) as po, \
             tc.tile_pool(name="ps_o", bufs=1, space="PSUM") as ps_o:
            o_ps = [ps_o.tile([128, QL], f32, tag=f"o{s}", name=f"o_ps{s}")
                    for s in range(4)]
            n_blk = NHC * N_CORES
            bi = 0
            for part in range(2):
                for cp in range(N_CORES):
                    for hh in range(2):
                        h = part * 2 + hh
                        g = NHC * cp + h
                        agt = po.tile([128, QL], f16, tag="agt")
                        nc.sync.dma_start(
                            agt[:],
                            agout_ab[part][cp * 256 + hh * 128:
                                           cp * 256 + (hh + 1) * 128, :])
                        st, sp = (bi == 0), (bi == n_blk - 1)
                        for s in range(4):
                            nc.tensor.matmul(o_ps[s][:],
                                             agt[:, s * 128:(s + 1) * 128],
                                             wot_tiles[g][:], start=st, stop=sp)
                        bi += 1
            for s in range(4):
                osb = po.tile([128, QL], f32, tag="osb")
                nc.vector.tensor_copy(osb[:], o_ps[s][:])
                nc.sync.dma_start(OUT[s * 128:(s + 1) * 128, :], osb[:])

        pwot_cm.__exit__(None, None, None)
        pqkv_cm.__exit__(None, None, None)
        pqdq_cm.__exit__(None, None, None)
        pconst_cm.__exit__(None, None, None)

    nc.compile()
    return nc


def _host_prep(inputs):
    hid = np.asarray(inputs["hidden_states"], dtype=np.float32)[0]   # [512, 4096]
    wq = np.asarray(inputs["wq"], dtype=np.float32)
    wk = np.asarray(inputs["wk"], dtype=np.float32)
    wv = np.asarray(inputs["wv"], dtype=np.float32)
    wo = np.asarray(inputs["wo"], dtype=np.float32)
    pk = np.asarray(inputs["past_key"], dtype=np.float32)[0]         # [8, 3584, 128]
    pv = np.asarray(inputs["past_value"], dtype=np.float32)[0]
    pos = np.asarray(inputs["position_ids"])[0].astype(np.float32)   # [512]

    hidT = np.ascontiguousarray(hid.T)
    inv_freq = np.float32(1.0) / (
        np.float32(10000.0) ** (np.arange(0, HD, 2, dtype=np.float32)
                                / np.float32(HD)))
    freqs = (pos[:, None] * inv_freq[None, :]).astype(np.float32)    # [512, 64]
    emb = np.concatenate([freqs, freqs], axis=-1).astype(np.float64)
    cosT = np.ascontiguousarray(np.cos(emb).astype(np.float32).T)    # [128, 512]
    sinT = np.ascontiguousarray(np.sin(emb).astype(np.float32).T)
    sinTs = sinT.copy()
    sinTs[0:64] *= np.float32(-1.0)
    mask = (np.arange(QL)[:, None] <= np.arange(QL)[None, :]).astype(np.float32)
    # device layout: [partition, (mask-chunk, seq)]
    maskp = np.ascontiguousarray(
        mask.reshape(4, 128, QL).transpose(1, 0, 2).reshape(128, 4 * QL)
    ).astype(np.float16)
    ones = np.ones((HD, 1), np.float16)

    hidT16 = hidT.astype(np.float16)
    in_maps = []
    for c in range(N_CORES):
        in_maps.append({
            "hidt": hidT16,
            "wqt": np.ascontiguousarray(wq[c * 512:(c + 1) * 512, :].T).astype(np.float16),
            "wkt": np.ascontiguousarray(wk[c * 128:(c + 1) * 128, :].T).astype(np.float16),
            "wvt": np.ascontiguousarray(wv[c * 128:(c + 1) * 128, :].T).astype(np.float16),
            "wot": np.ascontiguousarray(wo[c * 512:(c + 1) * 512, :].T).astype(np.float16),
            "pkt": np.ascontiguousarray(pk[c].T),
            "pv": np.ascontiguousarray(pv[c]),
            "cost": cosT,
            "sints": sinTs,
            "maskp": maskp,
            "ones": ones,
        })
    return in_maps


def _run(inputs, trace=False):
    from concourse.bass_utils import run_bass_kernel_spmd
    if "nc" not in _CACHE:
        _CACHE["nc"] = _build()
    nc = _CACHE["nc"]
    in_maps = _host_prep(inputs)
    res = run_bass_kernel_spmd(nc, in_maps, list(range(N_CORES)), trace=trace)
    out = np.concatenate([res.results[c]["out"] for c in range(N_CORES)], axis=1)
    return out.reshape(1, QL, HID).astype(np.float32), res


def kernel(**inputs) -> np.ndarray:
    out, _ = _run(inputs, trace=False)
    return out
